# revision 1
# baseline (speedup 1.0000x reference)
"""CrossAttnBlock on 8 trn2 NeuronCores.

Sharding: core c -> batch b=c//4, head-quad hq=c%4 (4 of 16 heads).
Attention is Megatron-sliced over heads; the out-projection partial sums
are combined with a bf16 ReduceScatter over each batch's 4 cores, which
also hands every core exactly its k-quarter for the row-parallel
residual+LN+FFN tail.  Host folds LN affine params into the projection
weights (exact) and computes the scalar gates.
"""
import sys
import numpy as np

sys.path.insert(0, "/opt/trn_rl_repo")

import ml_dtypes  # noqa: E402
import concourse.bass as bass  # noqa: E402
import concourse.mybir as mybir  # noqa: E402
import concourse.tile as tile  # noqa: E402
from concourse import bacc  # noqa: E402
from concourse import bass_utils  # noqa: E402

F32 = mybir.dt.float32
BF16 = mybir.dt.bfloat16
AF = mybir.ActivationFunctionType
OP = mybir.AluOpType

D = 1024
H = 16
HD = 64
B = 2
K = 1024
S = 4096
EPS = 1e-5
N_CORES = 8
KQ = K // 4          # rows per core after ReduceScatter
HC = 4               # heads per core
DH = HC * HD         # ctx dims per core (256)
P = 128
DC = D // P          # 8 D-chunks
D2 = 2 * D

_CACHE = {}


def _ln_tile(nc, pool, xt, n_free, gate_col=None):
    """LN over free dim of xt [128, n_free] f32 -> bf16 tile (returned).
    gate_col: optional [128,1] f32 to fold into the scale."""
    n_sub = (n_free + 511) // 512
    st = pool.tile([P, n_sub, 6], F32, tag="ln_st")
    xs = xt.rearrange("p (s f) -> p s f", s=n_sub)
    for i in range(n_sub):
        nc.vector.bn_stats(out=st[:, i, :], in_=xs[:, i, :])
    mv = pool.tile([P, 2], F32, tag="ln_mv")
    nc.vector.bn_aggr(out=mv, in_=st[:, :, :])
    rs = pool.tile([P, 1], F32, tag="ln_rs")
    nc.scalar.activation(rs, mv[:, 1:2], AF.Sqrt, bias=nc._eps_t[:, :], scale=1.0)
    nc.vector.reciprocal(rs, rs)
    if gate_col is not None:
        nc.vector.tensor_tensor(out=rs, in0=rs, in1=gate_col, op=OP.mult)
    xn = pool.tile([P, n_free], BF16, tag="ln_out")
    nc.vector.tensor_scalar(out=xn, in0=xt, scalar1=mv[:, 0:1], scalar2=rs,
                            op0=OP.subtract, op1=OP.mult)
    return xn


def _build_nc(taps=False):
    nc = bacc.Bacc("TRN2", target_bir_lowering=False, debug=False,
                   num_devices=N_CORES)

    dt_in = {}
    def din(name, shape):
        dt_in[name] = nc.dram_tensor(name, shape, F32, kind="ExternalInput")
        return dt_in[name]

    q_d = din("q", [K, D])
    kv_d = din("kv", [S, D])
    ab_d = din("ab", [K, S])
    ob_d = din("ob", [K, S])
    gate_d = din("gate", [K, 1])
    g1_d = din("g1", [K, 1])
    g2_d = din("g2", [K, 1])
    qres_d = din("q_res", [KQ, D])
    def dbf(name, shape):
        dt_in[name] = nc.dram_tensor(name, shape, BF16, kind="ExternalInput")
        return dt_in[name]
    wq_d = dbf("wq", [D, DH])
    wk_d = dbf("wk", [D, DH])
    wv_d = dbf("wv", [D, DH])
    bq_d = dbf("bq", [1, DH])
    bk_d = dbf("bk", [1, DH])
    bv_d = dbf("bv", [1, DH])
    grow_d = dbf("growb", [1, K])
    wo_d = dbf("wo", [DH, D])
    bo_d = din("bo", [1, D])
    w1_d = dbf("w1", [D, D2])
    b1_d = dbf("b1", [1, D2])
    w2_d = dbf("w2", [D2, D])
    b2_d = din("b2", [1, D])
    out_d = nc.dram_tensor("xq", [KQ, D], F32, kind="ExternalOutput")

    rs_out = nc.dram_tensor("rs_out", [KQ, D], BF16)
    tap = {}
    if taps:
        tap["qpT"] = nc.dram_tensor("t_qpT", [P, 2, K], BF16, kind="ExternalOutput")
        tap["kpT"] = nc.dram_tensor("t_kpT", [P, 2, S], BF16, kind="ExternalOutput")
        tap["vp"] = nc.dram_tensor("t_vp", [P, 32, HC * 65], BF16, kind="ExternalOutput")
        tap["cbt"] = nc.dram_tensor("t_cbt", [P, S // P, K], BF16, kind="ExternalOutput")
        tap["at0"] = nc.dram_tensor("t_at0", [P, 512], BF16, kind="ExternalOutput")
        tap["rr"] = nc.dram_tensor("t_rr", [1, 512], F32, kind="ExternalOutput")
        tap["ctx"] = nc.dram_tensor("t_ctx", [P, 2, K], BF16, kind="ExternalOutput")
        tap["y"] = nc.dram_tensor("t_y", [P, K // P, D], BF16, kind="ExternalOutput")
        tap["rs"] = nc.dram_tensor("t_rs", [P, 2, D], BF16, kind="ExternalOutput")
        tap["x"] = nc.dram_tensor("t_x", [P, 2, D], F32, kind="ExternalOutput")
        tap["h1"] = nc.dram_tensor("t_h1", [P, D2 // P, KQ], BF16, kind="ExternalOutput")

    groups = [[0, 1, 2, 3], [4, 5, 6, 7]]

    with tile.TileContext(nc) as tc:
        with (
            tc.tile_pool(name="const", bufs=1) as cpool,
            tc.tile_pool(name="persist", bufs=1) as pp,
            tc.tile_pool(name="dram", bufs=1, space="DRAM") as dpool,
        ):
            # ---- constants ----
            eps_t = cpool.tile([P, 1], F32)
            nc.vector.memset(eps_t, EPS)
            nc._eps_t = eps_t
            ones_row = cpool.tile([1, 512], BF16)
            nc.vector.memset(ones_row, 1.0)
            ones64 = cpool.tile([1, 64], F32)
            nc.vector.memset(ones64, 1.0)
            gsb = cpool.tile([P, DC], F32)
            nc.sync.dma_start(out=gsb, in_=gate_d.ap().rearrange(
                "(t p) o -> p (t o)", p=P))
            g1sb = cpool.tile([P, DC], F32)
            nc.sync.dma_start(out=g1sb, in_=g1_d.ap().rearrange(
                "(t p) o -> p (t o)", p=P))
            g2sb = cpool.tile([P, DC], F32)
            nc.sync.dma_start(out=g2sb, in_=g2_d.ap().rearrange(
                "(t p) o -> p (t o)", p=P))
            grow_bf = cpool.tile([1, K], BF16)
            nc.sync.dma_start(out=grow_bf, in_=grow_d[:, :])
            bq_bf = cpool.tile([1, DH], BF16)
            nc.sync.dma_start(out=bq_bf, in_=bq_d[:, :])
            bk_bf = cpool.tile([1, DH], BF16)
            nc.sync.dma_start(out=bk_bf, in_=bk_d[:, :])
            bv_bf = cpool.tile([1, DH], BF16)
            nc.sync.dma_start(out=bv_bf, in_=bv_d[:, :])
            b1_bf = cpool.tile([1, D2], BF16)
            nc.sync.dma_start(out=b1_bf, in_=b1_d[:, :])

            # ---- persistent activation tensors ----
            qpT = pp.tile([P, 2, K], BF16)       # [2 heads x 64, hp, k]
            kpT = pp.tile([P, 2, S], BF16)
            vp = pp.tile([P, 32, HC * 65], BF16)  # [s%128, s//128, h*65+(hd|one)]
            ctxT = pp.tile([P, 2, K], BF16)

            # ================= q: LN+gate -> transpose -> projection ====
            with (
                tc.tile_pool(name="projw", bufs=1) as wpool,
                tc.tile_pool(name="psA", bufs=4, space="PSUM") as psA,
            ):
                wq_bf = wpool.tile([P, DC, DH], BF16)
                nc.sync.dma_start(out=wq_bf, in_=wq_d.ap().rearrange(
                    "(c p) n -> p c n", p=P))
                wk_bf = wpool.tile([P, DC, DH], BF16)
                nc.sync.dma_start(out=wk_bf, in_=wk_d.ap().rearrange(
                    "(c p) n -> p c n", p=P))
                wv_bf = wpool.tile([P, DC, DH], BF16)
                nc.sync.dma_start(out=wv_bf, in_=wv_d.ap().rearrange(
                    "(c p) n -> p c n", p=P))

                with (
                    tc.tile_pool(name="lnq_big", bufs=1) as qbig,
                    tc.tile_pool(name="lnq", bufs=3) as lpool,
                ):
                    qT = qbig.tile([P, DC, K], BF16, tag="qT")
                    for t in range(K // P):
                        qt = lpool.tile([P, D], F32, tag="ln_in")
                        nc.sync.dma_start(out=qt, in_=q_d[t * P:(t + 1) * P, :])
                        qn = _ln_tile(nc, lpool, qt, D, gate_col=gsb[:, t:t + 1])
                        nc.scalar.dma_start_transpose(
                            qT[:, :, t * P:(t + 1) * P], qn[:, :])
                    # q projection: psum[2hd, 512 tok]
                    for hp in range(2):
                        for tb in range(K // 512):
                            ps = psA.tile([P, 512], F32, tag="mm")
                            for dc in range(DC):
                                nc.tensor.matmul(
                                    ps[:, :],
                                    wq_bf[:, dc, hp * P:(hp + 1) * P],
                                    qT[:, dc, tb * 512:(tb + 1) * 512],
                                    start=(dc == 0), stop=False)
                            nc.tensor.matmul(
                                ps[:, :], bq_bf[0:1, hp * P:(hp + 1) * P],
                                grow_bf[0:1, tb * 512:(tb + 1) * 512],
                                start=False, stop=True)
                            nc.scalar.activation(
                                qpT[:, hp, tb * 512:(tb + 1) * 512], ps[:, :],
                                AF.Identity)

                # ============ kv: LN -> transpose -> k/v projections =====
                with (
                    tc.tile_pool(name="lnkv_big", bufs=1) as kbig,
                    tc.tile_pool(name="lnkv", bufs=3) as lpool,
                ):
                    kvT = kbig.tile([P, DC, S], BF16, tag="kvT")
                    vpT = kbig.tile([P, 2, S], BF16, tag="vpT")
                    for t in range(S // P):
                        xt = lpool.tile([P, D], F32, tag="ln_in")
                        nc.sync.dma_start(out=xt, in_=kv_d[t * P:(t + 1) * P, :])
                        xn = _ln_tile(nc, lpool, xt, D)
                        nc.scalar.dma_start_transpose(
                            kvT[:, :, t * P:(t + 1) * P], xn[:, :])
                    for hp in range(2):
                        for sb in range(S // 512):
                            for dst, w_bf, b_bf in ((kpT, wk_bf, bk_bf),
                                                    (vpT, wv_bf, bv_bf)):
                                ps = psA.tile([P, 512], F32, tag="mm")
                                for dc in range(DC):
                                    nc.tensor.matmul(
                                        ps[:, :],
                                        w_bf[:, dc, hp * P:(hp + 1) * P],
                                        kvT[:, dc, sb * 512:(sb + 1) * 512],
                                        start=(dc == 0), stop=False)
                                nc.tensor.matmul(
                                    ps[:, :], b_bf[0:1, hp * P:(hp + 1) * P],
                                    ones_row[0:1, :],
                                    start=False, stop=True)
                                nc.scalar.activation(
                                    dst[:, hp, sb * 512:(sb + 1) * 512],
                                    ps[:, :], AF.Identity)
                    # vp natural layout [s, hd] per head + ones column.
                    # (batched dma transpose needs a full-128-partition
                    # source, so transpose per head-pair then split.)
                    for hp in range(2):
                        vps = lpool.tile([P, 32, P], BF16, tag="vps")
                        nc.scalar.dma_start_transpose(vps[:, :, :],
                                                       vpT[:, hp, :])
                        for half in range(2):
                            h = hp * 2 + half
                            nc.vector.tensor_copy(
                                vp[:, :, h * 65:h * 65 + 64],
                                vps[:, :, half * 64:half * 64 + 64])
                            nc.vector.memset(
                                vp[:, :, h * 65 + 64:h * 65 + 65], 1.0)
                if taps:
                    nc.sync.dma_start(out=tap["qpT"].ap(), in_=qpT[:, :, :])
                    nc.sync.dma_start(out=tap["kpT"].ap(), in_=kpT[:, :, :])
                    nc.sync.dma_start(out=tap["vp"].ap(), in_=vp[:, :, :])

            # ============ preload ffn weights (overlap with attention) ===
            wpre_cm = tc.tile_pool(name="wpre", bufs=1)
            wpre = wpre_cm.__enter__()
            wo_bf = wpre.tile([P, 2, D], BF16)
            nc.sync.dma_start(out=wo_bf, in_=wo_d.ap().rearrange(
                "(c p) n -> p c n", p=P))
            w1_bf = wpre.tile([P, DC, D2], BF16)
            nc.sync.dma_start(out=w1_bf, in_=w1_d.ap().rearrange(
                "(c p) n -> p c n", p=P))

            # ====== ET = exp(gate*(c1*AB + c2*OB)), transposed ===========
            with tc.tile_pool(name="cbt", bufs=1) as cbtp:
                ET = cbtp.tile([P, S // P, K], BF16)
                with tc.tile_pool(name="cbs", bufs=3) as cbp:
                    for kc in range(K // P):
                        for sg in range(S // 1024):
                            ssl = slice(sg * 1024, (sg + 1) * 1024)
                            abt = cbp.tile([P, 1024], F32, tag="ab")
                            nc.gpsimd.dma_start(
                                out=abt, in_=ab_d[kc * P:(kc + 1) * P, ssl])
                            obt = cbp.tile([P, 1024], F32, tag="ob")
                            nc.gpsimd.dma_start(
                                out=obt, in_=ob_d[kc * P:(kc + 1) * P, ssl])
                            t1 = cbp.tile([P, 1024], F32, tag="t1")
                            nc.vector.tensor_scalar_mul(
                                out=t1, in0=obt, scalar1=g2sb[:, kc:kc + 1])
                            cb = cbp.tile([P, 1024], F32, tag="cb")
                            nc.vector.scalar_tensor_tensor(
                                out=cb, in0=abt, scalar=g1sb[:, kc:kc + 1],
                                in1=t1, op0=OP.mult, op1=OP.add)
                            eb = cbp.tile([P, 1024], BF16, tag="eb")
                            nc.scalar.activation(eb, cb, AF.Exp)
                            nc.sync.dma_start_transpose(
                                ET[:, sg * 8:(sg + 1) * 8, kc * P:(kc + 1) * P],
                                eb[:, :])
                if taps:
                    nc.sync.dma_start(out=tap["cbt"].ap(), in_=CBT[:, :, :])

                # ======================= attention =======================
                with (
                    tc.tile_pool(name="att", bufs=4) as apool,
                    tc.tile_pool(name="psS", bufs=4, space="PSUM") as psS,
                    tc.tile_pool(name="psPV", bufs=1, space="PSUM") as psPV,
                    tc.tile_pool(name="psRR", bufs=1, space="PSUM") as psRR,
                ):
                    for h in range(HC):
                        pb = (h % 2) * 64
                        hp = h // 2
                        pvs = [psPV.tile([65, 512], F32, tag=f"pv{kb}",
                                         name=f"pv_{h}_{kb}")
                               for kb in range(K // 512)]
                        for sc in range(S // P):
                            for kb in range(K // 512):
                                pv = pvs[kb]
                                sps = psS.tile([P, 512], F32, tag="sc")
                                nc.tensor.matmul(
                                    sps[:, :],
                                    kpT[pb:pb + 64, hp, sc * P:(sc + 1) * P],
                                    qpT[pb:pb + 64, hp, kb * 512:(kb + 1) * 512],
                                    start=True, stop=True)
                                eq = apool.tile([P, 512], BF16, tag="eq")
                                nc.scalar.activation(eq, sps[:, :], AF.Exp)
                                at = apool.tile([P, 512], BF16, tag="at")
                                nc.vector.tensor_tensor(
                                    out=at, in0=eq,
                                    in1=ET[:, sc, kb * 512:(kb + 1) * 512],
                                    op=OP.mult)
                                if taps and h == 0 and kb == 0 and sc == 0:
                                    nc.sync.dma_start(out=tap["at0"].ap(), in_=at[:, :])
                                nc.tensor.matmul(
                                    pv[:, :],
                                    vp[:, sc, h * 65:(h + 1) * 65],
                                    at[:, :],
                                    start=(sc == 0), stop=(sc == S // P - 1))
                        for kb in range(K // 512):
                            pv = pvs[kb]
                            rr = apool.tile([1, 512], F32, tag="rr")
                            nc.vector.reciprocal(rr, pv[64:65, :])
                            if taps and h == 0 and kb == 0:
                                nc.sync.dma_start(out=tap["rr"].ap(), in_=rr[:, :])
                            rrb = psRR.tile([64, 512], F32, tag="rrb")
                            nc.tensor.matmul(rrb[:, :], ones64[:, :], rr[:, :],
                                             start=True, stop=True)
                            rrs = apool.tile([64, 512], F32, tag="rrs")
                            nc.scalar.activation(rrs, rrb[:, :], AF.Identity)
                            nc.vector.tensor_tensor(
                                out=ctxT[pb:pb + 64, hp, kb * 512:(kb + 1) * 512],
                                in0=pv[0:64, :], in1=rrs, op=OP.mult)

            if taps:
                nc.sync.dma_start(out=tap["ctx"].ap(), in_=ctxT[:, :, :])
            # ============== out-proj partial + ReduceScatter =============
            with (
                tc.tile_pool(name="ffn", bufs=1) as fp,
                tc.tile_pool(name="fstream", bufs=3) as fs,
                tc.tile_pool(name="psF", bufs=3, space="PSUM") as psF,
                tc.tile_pool(name="psH", bufs=3, space="PSUM") as psH,
            ):
                y_sb = fp.tile([P, K // P, D], BF16)
                for kt in range(K // P):
                    for db in range(D // 512):
                        ps = psF.tile([P, 512], F32, tag="y")
                        for cc in range(2):
                            nc.tensor.matmul(
                                ps[:, :],
                                ctxT[:, cc, kt * P:(kt + 1) * P],
                                wo_bf[:, cc, db * 512:(db + 1) * 512],
                                start=(cc == 0), stop=(cc == 1))
                        nc.scalar.activation(
                            y_sb[:, kt, db * 512:(db + 1) * 512], ps[:, :],
                            AF.Identity)
                if taps:
                    nc.sync.dma_start(out=tap["y"].ap(), in_=y_sb[:, :, :])
                rs_in = dpool.tile([K, D], BF16)
                nc.sync.dma_start(
                    out=rs_in.rearrange("(t p) d -> p t d", p=P), in_=y_sb[:, :, :])
                nc.gpsimd.collective_compute(
                    "ReduceScatter", OP.add, replica_groups=groups,
                    ins=[rs_in.opt()], outs=[rs_out.ap().opt()])

                # ======= residual + LN_f + FFN on my KQ rows ============
                w2_bf = fp.tile([P, D2 // P, D], BF16)
                nc.sync.dma_start(out=w2_bf, in_=w2_d.ap().rearrange(
                    "(c p) n -> p c n", p=P))
                bo_bc = fp.tile([P, D], F32)
                nc.sync.dma_start(out=bo_bc, in_=bass.AP(
                    tensor=bo_d, offset=0, ap=[[0, P], [1, D]]))
                b2_bc = fp.tile([P, D], F32)
                nc.sync.dma_start(out=b2_bc, in_=bass.AP(
                    tensor=b2_d, offset=0, ap=[[0, P], [1, D]]))

                rs_sb = fp.tile([P, 2, D], BF16)
                nc.sync.dma_start(out=rs_sb,
                                  in_=rs_out.ap().rearrange("(t p) d -> p t d", p=P))
                x_sb = fp.tile([P, 2, D], F32)
                xfT = fp.tile([P, DC, KQ], BF16)
                if taps:
                    nc.sync.dma_start(out=tap["rs"].ap(), in_=rs_sb[:, :, :])
                for kt in range(KQ // P):
                    qr = fs.tile([P, D], F32, tag="qr")
                    nc.sync.dma_start(out=qr, in_=qres_d[kt * P:(kt + 1) * P, :])
                    nc.vector.tensor_tensor(out=x_sb[:, kt, :], in0=qr,
                                            in1=rs_sb[:, kt, :], op=OP.add)
                    nc.vector.tensor_tensor(out=x_sb[:, kt, :], in0=x_sb[:, kt, :],
                                            in1=bo_bc, op=OP.add)
                    xn = _ln_tile(nc, fs, x_sb[:, kt, :], D)
                    nc.scalar.dma_start_transpose(
                        xfT[:, :, kt * P:(kt + 1) * P], xn[:, :])

                if taps:
                    nc.sync.dma_start(out=tap["x"].ap(), in_=x_sb[:, :, :])
                h1T = fp.tile([P, D2 // P, KQ], BF16)
                for hc in range(D2 // P):
                    ps = psH.tile([P, KQ], F32, tag="h1")
                    for dc in range(DC):
                        nc.tensor.matmul(
                            ps[:, :], w1_bf[:, dc, hc * P:(hc + 1) * P],
                            xfT[:, dc, :], start=(dc == 0), stop=False)
                    nc.tensor.matmul(ps[:, :], b1_bf[0:1, hc * P:(hc + 1) * P],
                                     ones_row[0:1, 0:KQ], start=False, stop=True)
                    nc.scalar.activation(h1T[:, hc, :], ps[:, :], AF.Gelu)

                if taps:
                    nc.sync.dma_start(out=tap["h1"].ap(), in_=h1T[:, :, :])
                o_sb = fp.tile([P, 2, D], F32)
                for kt in range(KQ // P):
                    for db in range(D // 512):
                        ps = psF.tile([P, 512], F32, tag="y")
                        for hc in range(D2 // P):
                            nc.tensor.matmul(
                                ps[:, :], h1T[:, hc, kt * P:(kt + 1) * P],
                                w2_bf[:, hc, db * 512:(db + 1) * 512],
                                start=(hc == 0), stop=(hc == D2 // P - 1))
                        sl = slice(db * 512, (db + 1) * 512)
                        nc.vector.tensor_tensor(out=o_sb[:, kt, sl], in0=ps[:, :],
                                                in1=x_sb[:, kt, sl], op=OP.add)
                        nc.vector.tensor_tensor(out=o_sb[:, kt, sl],
                                                in0=o_sb[:, kt, sl],
                                                in1=b2_bc[:, sl], op=OP.add)
                nc.sync.dma_start(
                    out=out_d.ap().rearrange("(t p) d -> p t d", p=P),
                    in_=o_sb[:, :, :])
            wpre_cm.__exit__(None, None, None)

    nc.compile()
    return nc


def _get_nc(taps=False):
    key = "nc_taps" if taps else "nc"
    if key not in _CACHE:
        _CACHE[key] = _build_nc(taps=taps)
    return _CACHE[key]


def _softplus(x):
    return float(np.log1p(np.exp(np.float64(x))))


def kernel(**inputs):
    f = lambda name: np.ascontiguousarray(np.asarray(inputs[name], np.float32))
    q = f("q"); kv = f("kv"); ab = f("attn_bias"); ob = f("obs_bias")
    density = f("density")
    c1 = _softplus(inputs["dist_raw"])
    c2 = _softplus(inputs["obs_raw"])
    tg = float(np.tanh(np.float64(np.asarray(inputs["dens_raw"], np.float64))))
    gate = (1.0 + tg * density).astype(np.float32)       # [B, K]

    ln_q_w = f("ln_q_w"); ln_q_b = f("ln_q_b")
    ln_kv_w = f("ln_kv_w"); ln_kv_b = f("ln_kv_b")
    ln_f_w = f("ln_f_w"); ln_f_b = f("ln_f_b")
    scale = np.float32(HD ** -0.5)
    wq = scale * ln_q_w[:, None] * f("wq")
    bq = scale * (ln_q_b @ f("wq") + f("bq"))
    wk = ln_kv_w[:, None] * f("wk"); bk = ln_kv_b @ f("wk") + f("bk")
    wv = ln_kv_w[:, None] * f("wv"); bv = ln_kv_b @ f("wv") + f("bv")
    w1 = ln_f_w[:, None] * f("w1"); b1 = ln_f_b @ f("w1") + f("b1")
    wo = f("wo"); bo = f("bo"); w2 = f("w2"); b2 = f("b2")

    cont = np.ascontiguousarray
    bf = lambda a: np.ascontiguousarray(np.asarray(a, dtype=ml_dtypes.bfloat16))
    in_maps = []
    for c in range(N_CORES):
        b, hq = divmod(c, 4)
        hs = slice(hq * DH, (hq + 1) * DH)
        ks = slice(hq * KQ, (hq + 1) * KQ)
        in_maps.append({
            "q": cont(q[b]), "kv": cont(kv[b]),
            "ab": cont(ab[b]), "ob": cont(ob[b]),
            "gate": cont(gate[b][:, None]),
            "g1": cont((gate[b] * c1)[:, None]),
            "g2": cont((gate[b] * c2)[:, None]),
            "growb": bf(gate[b][None, :]),
            "q_res": cont(q[b, ks]),
            "wq": bf(wq[:, hs]), "wk": bf(wk[:, hs]), "wv": bf(wv[:, hs]),
            "bq": bf(bq[None, hs]), "bk": bf(bk[None, hs]),
            "bv": bf(bv[None, hs]),
            "wo": bf(wo[hs, :]), "bo": cont(bo[None, :]),
            "w1": bf(w1), "b1": bf(b1[None, :]),
            "w2": bf(w2), "b2": cont(b2[None, :]),
        })

    global _last_in_maps
    _last_in_maps = in_maps
    nc = _get_nc()
    res = bass_utils.run_bass_kernel_spmd(
        nc, in_maps, core_ids=list(range(N_CORES)))
    out = np.empty((B, K, D), np.float32)
    for c in range(N_CORES):
        b, hq = divmod(c, 4)
        out[b, hq * KQ:(hq + 1) * KQ, :] = res.results[c]["xq"]
    return out



# revision 6
# speedup vs baseline: 1.6484x; 1.6484x over previous
"""CrossAttnBlock on 8 trn2 NeuronCores.

Sharding: core c -> batch b=c//4, rank r=c%4 within the batch group.
Attention is Megatron-sliced over heads (4 of 16 per core); the
out-projection partial sums are combined with TWO chunked bf16
ReduceScatters (one per K-half), each launched as soon as its half of
the attention context is ready, so the first fully overlaps the second
half of attention and the FFN on the first half overlaps the second RS.
After the RS, core r owns k-rows {r*128..+128} of each half; the host
maps them back.

Key optimizations vs the single-RS baseline:
- The combined attention bias term exp(gate*(c1*ab + c2*ob)) is computed
  on the host and streamed in as a [S, K] bf16 tensor (8.4 MB vs 33.6 MB
  f32 + on-device exp + transpose), block-streamed inside the attention
  loop, so the 320 us serial bias phase is gone entirely.
- QK^T matmuls for the two heads of a pair run concurrently in the PE
  array via tile_position row-packing (contraction is only 64).
- Softmax exp runs at FD=1024 over two PSUM banks per ACTIVATE.
- DMA queues are split (gpsimd: bulk loads, sync: transposes + stores,
  scalar: bias stream + tail weights) to avoid head-of-line blocking.
"""
import sys
import numpy as np

sys.path.insert(0, "/opt/trn_rl_repo")

import ml_dtypes  # noqa: E402
import concourse.bass as bass  # noqa: E402
import concourse.mybir as mybir  # noqa: E402
import concourse.tile as tile  # noqa: E402
from concourse import bacc  # noqa: E402
from concourse import bass_utils  # noqa: E402

F32 = mybir.dt.float32
BF16 = mybir.dt.bfloat16
AF = mybir.ActivationFunctionType
OP = mybir.AluOpType

D = 1024
H = 16
HD = 64
B = 2
K = 1024
S = 4096
EPS = 1e-5
N_CORES = 8
KQ = K // 4          # rows per core after the two ReduceScatters
HC = 4               # heads per core
DH = HC * HD         # ctx dims per core (256)
P = 128
DC = D // P          # 8 D-chunks
D2 = 2 * D

_CACHE = {}


def _ln_tile(nc, pool, xt, n_free, gate_col=None):
    """LN over free dim of xt [128, n_free] f32 -> bf16 tile (returned).
    gate_col: optional [128,1] f32 to fold into the scale."""
    n_sub = (n_free + 511) // 512
    st = pool.tile([P, n_sub, 6], F32, tag="ln_st")
    xs = xt.rearrange("p (s f) -> p s f", s=n_sub)
    for i in range(n_sub):
        nc.vector.bn_stats(out=st[:, i, :], in_=xs[:, i, :])
    mv = pool.tile([P, 2], F32, tag="ln_mv")
    nc.vector.bn_aggr(out=mv, in_=st[:, :, :])
    rs = pool.tile([P, 1], F32, tag="ln_rs")
    nc.scalar.activation(rs, mv[:, 1:2], AF.Sqrt, bias=nc._eps_t[:, :], scale=1.0)
    nc.vector.reciprocal(rs, rs)
    if gate_col is not None:
        nc.vector.tensor_tensor(out=rs, in0=rs, in1=gate_col, op=OP.mult)
    xn = pool.tile([P, n_free], BF16, tag="ln_out")
    nc.vector.tensor_scalar(out=xn, in0=xt, scalar1=mv[:, 0:1], scalar2=rs,
                            op0=OP.subtract, op1=OP.mult)
    return xn


def _build_nc():
    nc = bacc.Bacc("TRN2", target_bir_lowering=False, debug=False,
                   num_devices=N_CORES)

    def din(name, shape, dt=F32):
        return nc.dram_tensor(name, shape, dt, kind="ExternalInput")

    q_d = din("q", [K, D])
    kv_d = din("kv", [S, D])
    et_d = din("et", [S, K], BF16)
    gate_d = din("gate", [K, 1])
    qres_d = din("q_res", [KQ, D])
    wq_d = din("wq", [D, DH], BF16)
    wk_d = din("wk", [D, DH], BF16)
    wv_d = din("wv", [D, DH], BF16)
    bq_d = din("bq", [1, DH], BF16)
    bk_d = din("bk", [1, DH], BF16)
    bv_d = din("bv", [1, DH], BF16)
    grow_d = din("growb", [1, K], BF16)
    wo_d = din("wo", [DH, D], BF16)
    w1_d = din("w1", [D, D2], BF16)
    b1_d = din("b1", [1, D2], BF16)
    w2_d = din("w2", [D2, D], BF16)
    b2_d = din("b2", [1, D], BF16)
    out_d = nc.dram_tensor("xq", [KQ, D], F32, kind="ExternalOutput")

    rs_out = [nc.dram_tensor(f"rs_out{i}", [P, D], BF16) for i in range(2)]
    groups = [[0, 1, 2, 3], [4, 5, 6, 7]]

    with tile.TileContext(nc) as tc:
        with (
            tc.tile_pool(name="const", bufs=1) as cpool,
            tc.tile_pool(name="persist", bufs=1) as pp,
            tc.tile_pool(name="wt", bufs=1) as wt,
            tc.tile_pool(name="dram", bufs=1, space="DRAM") as dpool,
        ):
            # ---- constants ----
            eps_t = cpool.tile([P, 1], F32)
            nc.vector.memset(eps_t, EPS)
            nc._eps_t = eps_t
            ones_row = cpool.tile([1, 512], BF16)
            nc.vector.memset(ones_row, 1.0)
            ones64 = cpool.tile([1, 64], F32)
            nc.vector.memset(ones64, 1.0)
            gsb = cpool.tile([P, DC], F32)
            nc.sync.dma_start(out=gsb, in_=gate_d.ap().rearrange(
                "(t p) o -> p (t o)", p=P))
            grow_bf = cpool.tile([1, K], BF16)
            nc.sync.dma_start(out=grow_bf, in_=grow_d[:, :])
            bq_bf = cpool.tile([1, DH], BF16)
            nc.sync.dma_start(out=bq_bf, in_=bq_d[:, :])
            bk_bf = cpool.tile([1, DH], BF16)
            nc.sync.dma_start(out=bk_bf, in_=bk_d[:, :])
            bv_bf = cpool.tile([1, DH], BF16)
            nc.sync.dma_start(out=bv_bf, in_=bv_d[:, :])
            b1_bf = cpool.tile([1, D2], BF16)
            nc.sync.dma_start(out=b1_bf, in_=b1_d[:, :])
            b2_bf = cpool.tile([1, D], BF16)
            nc.sync.dma_start(out=b2_bf, in_=b2_d[:, :])

            # ---- persistent activation tensors ----
            qpT = pp.tile([P, 2, K], BF16)       # [2 heads x 64, hp, k]
            kpT = pp.tile([P, 2, S], BF16)
            vp = pp.tile([P, 32, HC * 65], BF16)  # [s%128, s//128, h*65+(hd|one)]
            ctxT = pp.tile([P, 2, K], BF16)

            # ============ q: LN+gate -> transpose -> projection ==========
            with (
                tc.tile_pool(name="projw", bufs=1) as wpool,
                tc.tile_pool(name="psA", bufs=4, space="PSUM") as psA,
            ):
                wq_bf = wpool.tile([P, DC, DH], BF16)
                nc.sync.dma_start(out=wq_bf, in_=wq_d.ap().rearrange(
                    "(c p) n -> p c n", p=P))
                wk_bf = wpool.tile([P, DC, DH], BF16)
                nc.sync.dma_start(out=wk_bf, in_=wk_d.ap().rearrange(
                    "(c p) n -> p c n", p=P))
                wv_bf = wpool.tile([P, DC, DH], BF16)
                nc.sync.dma_start(out=wv_bf, in_=wv_d.ap().rearrange(
                    "(c p) n -> p c n", p=P))

                with (
                    tc.tile_pool(name="lnq_big", bufs=1) as qbig,
                    tc.tile_pool(name="lnq", bufs=3) as lpool,
                ):
                    qT = qbig.tile([P, DC, K], BF16, tag="qT")
                    for t in range(K // P):
                        qt = lpool.tile([P, D], F32, tag="ln_in")
                        nc.gpsimd.dma_start(out=qt,
                                            in_=q_d[t * P:(t + 1) * P, :])
                        qn = _ln_tile(nc, lpool, qt, D, gate_col=gsb[:, t:t + 1])
                        nc.sync.dma_start_transpose(
                            qT[:, :, t * P:(t + 1) * P], qn[:, :])
                    # q projection: psum[2hd, 512 tok]
                    for hp in range(2):
                        for tb in range(K // 512):
                            ps = psA.tile([P, 512], F32, tag="mm")
                            for dc in range(DC):
                                nc.tensor.matmul(
                                    ps[:, :],
                                    wq_bf[:, dc, hp * P:(hp + 1) * P],
                                    qT[:, dc, tb * 512:(tb + 1) * 512],
                                    start=(dc == 0), stop=False)
                            nc.tensor.matmul(
                                ps[:, :], bq_bf[0:1, hp * P:(hp + 1) * P],
                                grow_bf[0:1, tb * 512:(tb + 1) * 512],
                                start=False, stop=True)
                            nc.scalar.activation(
                                qpT[:, hp, tb * 512:(tb + 1) * 512], ps[:, :],
                                AF.Identity)

                # ====== kv: LN -> transpose -> k/v projections, by sg ======
                with (
                    tc.tile_pool(name="lnkv_big", bufs=2) as kbig,
                    tc.tile_pool(name="lnkv", bufs=3) as lpool,
                    tc.tile_pool(name="vps_p", bufs=2) as vpsp,
                ):
                    for h in range(HC):
                        nc.vector.memset(vp[:, :, h * 65 + 64:h * 65 + 65], 1.0)
                    for sg in range(4):
                        kvT = kbig.tile([P, DC, 1024], BF16, tag="kvT")
                        vpT = kbig.tile([P, 2, 1024], BF16, tag="vpT")
                        for t in range(8):
                            st_ = sg * 1024 + t * P
                            xt = lpool.tile([P, D], F32, tag="ln_in")
                            nc.gpsimd.dma_start(out=xt, in_=kv_d[st_:st_ + P, :])
                            xn = _ln_tile(nc, lpool, xt, D)
                            nc.sync.dma_start_transpose(
                                kvT[:, :, t * P:(t + 1) * P], xn[:, :])
                        for hp in range(2):
                            for sb_ in range(2):
                                ssl = slice(sb_ * 512, (sb_ + 1) * 512)
                                for dst, w_bf, b_bf in ((kpT, wk_bf, bk_bf),
                                                        (vpT, wv_bf, bv_bf)):
                                    ps = psA.tile([P, 512], F32, tag="mm")
                                    for dc in range(DC):
                                        nc.tensor.matmul(
                                            ps[:, :],
                                            w_bf[:, dc, hp * P:(hp + 1) * P],
                                            kvT[:, dc, ssl],
                                            start=(dc == 0), stop=False)
                                    nc.tensor.matmul(
                                        ps[:, :],
                                        b_bf[0:1, hp * P:(hp + 1) * P],
                                        ones_row[0:1, :],
                                        start=False, stop=True)
                                    if dst is kpT:
                                        osl = slice(sg * 1024 + sb_ * 512,
                                                    sg * 1024 + (sb_ + 1) * 512)
                                        nc.scalar.activation(
                                            kpT[:, hp, osl], ps[:, :],
                                            AF.Identity)
                                    else:
                                        nc.scalar.activation(
                                            vpT[:, hp, ssl], ps[:, :],
                                            AF.Identity)
                        # vp natural layout [s, hd] per head (+ ones column)
                        for hp in range(2):
                            vps = vpsp.tile([P, 8, P], BF16, tag="vps")
                            nc.sync.dma_start_transpose(vps[:, :, :],
                                                        vpT[:, hp, :])
                            for half in range(2):
                                h = hp * 2 + half
                                nc.vector.tensor_copy(
                                    vp[:, sg * 8:(sg + 1) * 8,
                                       h * 65:h * 65 + 64],
                                    vps[:, :, half * 64:half * 64 + 64])

            # tail weights: sync queue is idle once the projections are
            # done; these loads overlap the start of attention.
            wo_bf = wt.tile([P, 2, D], BF16)
            nc.sync.dma_start(out=wo_bf, in_=wo_d.ap().rearrange(
                "(c p) n -> p c n", p=P))
            w1_bf = wt.tile([P, DC, D2], BF16)
            nc.sync.dma_start(out=w1_bf, in_=w1_d.ap().rearrange(
                "(c p) n -> p c n", p=P))
            w2_bf = wt.tile([P, D2 // P, D], BF16)
            nc.sync.dma_start(out=w2_bf, in_=w2_d.ap().rearrange(
                "(c p) n -> p c n", p=P))
            qres_sb = wt.tile([P, 2, D], F32)
            nc.sync.dma_start(out=qres_sb, in_=qres_d.ap().rearrange(
                "(t p) d -> p t d", p=P))

            # ======================= attention ==========================
            # loop kb (k halves) -> sc (s tiles) -> hp (head pairs);
            # the bias-exp block streams from HBM per (kb, sc).  After each
            # kb, the out-proj partial for that k-half is computed and its
            # ReduceScatter launched (overlapping the next kb / the FFN).
            with (
                tc.tile_pool(name="att", bufs=4) as apool,
                tc.tile_pool(name="ets", bufs=4) as espool,
                tc.tile_pool(name="attr", bufs=2) as rpool,
                tc.tile_pool(name="ysb", bufs=1) as ypool,
                tc.tile_pool(name="psS", bufs=2, space="PSUM") as psS,
                tc.tile_pool(name="psPV", bufs=1, space="PSUM") as psPV,
            ):
                for kb in range(K // 512):
                    ksl = slice(kb * 512, (kb + 1) * 512)
                    pvs = [psPV.tile([65, 512], F32, tag=f"pv{h}",
                                     name=f"pv_{kb}_{h}")
                           for h in range(HC)]
                    for sc in range(S // P):
                        et_blk = espool.tile([P, 512], BF16, tag="et")
                        nc.scalar.dma_start(
                            out=et_blk,
                            in_=et_d.ap()[sc * P:(sc + 1) * P, ksl])
                        for hp in range(2):
                            sps = psS.tile([P, 1024], F32, tag="sc")
                            nc.tensor.matmul(
                                sps[:, 0:512],
                                kpT[0:64, hp, sc * P:(sc + 1) * P],
                                qpT[0:64, hp, ksl],
                                start=True, stop=True, tile_position=(0, 0))
                            nc.tensor.matmul(
                                sps[:, 512:1024],
                                kpT[64:128, hp, sc * P:(sc + 1) * P],
                                qpT[64:128, hp, ksl],
                                start=True, stop=True, tile_position=(64, 0))
                            eq = apool.tile([P, 1024], BF16, tag="eq")
                            nc.scalar.activation(eq, sps[:, :], AF.Exp)
                            at = apool.tile([P, 1024], BF16, tag="at")
                            et_v = et_blk[:, :].rearrange(
                                "p (o f) -> p o f", o=1).broadcast_to(
                                [P, 2, 512])
                            nc.vector.tensor_tensor(
                                out=at[:, :].rearrange("p (o f) -> p o f", o=2),
                                in0=eq[:, :].rearrange("p (o f) -> p o f", o=2),
                                in1=et_v, op=OP.mult)
                            he = hp * 2
                            ho = hp * 2 + 1
                            nc.tensor.matmul(
                                pvs[he][:, :],
                                vp[:, sc, he * 65:(he + 1) * 65],
                                at[:, 0:512],
                                start=(sc == 0), stop=(sc == S // P - 1))
                            nc.tensor.matmul(
                                pvs[ho][:, :],
                                vp[:, sc, ho * 65:(ho + 1) * 65],
                                at[:, 512:1024],
                                start=(sc == 0), stop=(sc == S // P - 1))
                    for h in range(HC):
                        pv = pvs[h]
                        pb = (h % 2) * 64
                        hp = h // 2
                        rr = rpool.tile([1, 512], F32, tag="rr")
                        nc.vector.reciprocal(rr, pv[64:65, :])
                        rps = psS.tile([P, 1024], F32, tag="sc")
                        nc.tensor.matmul(rps[0:64, 0:512], ones64[:, :],
                                         rr[:, :], start=True, stop=True)
                        rrs = rpool.tile([64, 512], F32, tag="rrs")
                        nc.scalar.activation(rrs, rps[0:64, 0:512], AF.Identity)
                        nc.vector.tensor_tensor(
                            out=ctxT[pb:pb + 64, hp, ksl],
                            in0=pv[0:64, :], in1=rrs, op=OP.mult)
                    # ---- out-proj partial for this k-half + ReduceScatter
                    y_sb = ypool.tile([P, 4, D], BF16, tag="y")
                    for tb in range(4):
                        tsl = slice(kb * 512 + tb * P, kb * 512 + (tb + 1) * P)
                        for db in range(D // 512):
                            dsl = slice(db * 512, (db + 1) * 512)
                            ps = psS.tile([P, 1024], F32, tag="sc")
                            for cc in range(2):
                                nc.tensor.matmul(
                                    ps[:, 0:512],
                                    ctxT[:, cc, tsl],
                                    wo_bf[:, cc, dsl],
                                    start=(cc == 0), stop=(cc == 1))
                            nc.vector.tensor_copy(y_sb[:, tb, dsl],
                                                  ps[:, 0:512])
                    rs_in = dpool.tile([512, D], BF16, tag=f"rsin{kb}")
                    nc.sync.dma_start(
                        out=rs_in.rearrange("(t p) d -> p t d", p=P),
                        in_=y_sb[:, :, :])
                    nc.gpsimd.collective_compute(
                        "ReduceScatter", OP.add, replica_groups=groups,
                        ins=[rs_in.opt()], outs=[rs_out[kb].ap().opt()])

            # ====== residual + LN_f + FFN per k-half (RS0 overlaps) =======
            with (
                tc.tile_pool(name="ffn", bufs=1) as fp,
                tc.tile_pool(name="fstream", bufs=3) as fs,
                tc.tile_pool(name="psF", bufs=3, space="PSUM") as psF,
                tc.tile_pool(name="psH", bufs=3, space="PSUM") as psH,
            ):
                x_sb = fp.tile([P, 2, D], F32)
                xfT = fp.tile([P, DC, KQ], BF16)
                h1T = fp.tile([P, D2 // P, KQ], BF16)
                o_sb = fp.tile([P, 2, D], F32)
                for kt in range(2):
                    rs_sb = fs.tile([P, D], BF16, tag="rs")
                    nc.sync.dma_start(out=rs_sb, in_=rs_out[kt].ap())
                    nc.vector.tensor_tensor(out=x_sb[:, kt, :], in0=rs_sb,
                                            in1=qres_sb[:, kt, :], op=OP.add)
                    xn = _ln_tile(nc, fs, x_sb[:, kt, :], D)
                    nc.sync.dma_start_transpose(
                        xfT[:, :, kt * P:(kt + 1) * P], xn[:, :])
                    for hc in range(D2 // P):
                        ps = psH.tile([P, P], F32, tag="h1")
                        for dc in range(DC):
                            nc.tensor.matmul(
                                ps[:, :], w1_bf[:, dc, hc * P:(hc + 1) * P],
                                xfT[:, dc, kt * P:(kt + 1) * P],
                                start=(dc == 0), stop=False)
                        nc.tensor.matmul(
                            ps[:, :], b1_bf[0:1, hc * P:(hc + 1) * P],
                            ones_row[0:1, 0:P], start=False, stop=True)
                        nc.scalar.activation(h1T[:, hc, kt * P:(kt + 1) * P],
                                             ps[:, :], AF.Gelu)
                    for db in range(D // 512):
                        dsl = slice(db * 512, (db + 1) * 512)
                        ps = psF.tile([P, 512], F32, tag="o")
                        for hc in range(D2 // P):
                            nc.tensor.matmul(
                                ps[:, :], h1T[:, hc, kt * P:(kt + 1) * P],
                                w2_bf[:, hc, dsl],
                                start=(hc == 0), stop=False)
                        nc.tensor.matmul(
                            ps[:, :], ones_row[0:1, 0:P],
                            b2_bf[0:1, dsl], start=False, stop=True)
                        nc.vector.tensor_tensor(out=o_sb[:, kt, dsl],
                                                in0=ps[:, :],
                                                in1=x_sb[:, kt, dsl],
                                                op=OP.add)
                    nc.sync.dma_start(
                        out=out_d.ap()[kt * P:(kt + 1) * P, :],
                        in_=o_sb[:, kt, :])

    nc.compile()
    return nc


def _get_nc():
    if "nc" not in _CACHE:
        _CACHE["nc"] = _build_nc()
    return _CACHE["nc"]


def _softplus(x):
    return float(np.log1p(np.exp(np.float64(x))))


def kernel(**inputs):
    f = lambda name: np.ascontiguousarray(np.asarray(inputs[name], np.float32))
    q = f("q"); kv = f("kv"); ab = f("attn_bias"); ob = f("obs_bias")
    density = f("density")
    c1 = _softplus(inputs["dist_raw"])
    c2 = _softplus(inputs["obs_raw"])
    tg = float(np.tanh(np.float64(np.asarray(inputs["dens_raw"], np.float64))))
    gate = (1.0 + tg * density).astype(np.float32)       # [B, K]

    ln_q_w = f("ln_q_w"); ln_q_b = f("ln_q_b")
    ln_kv_w = f("ln_kv_w"); ln_kv_b = f("ln_kv_b")
    ln_f_w = f("ln_f_w"); ln_f_b = f("ln_f_b")
    scale = np.float32(HD ** -0.5)
    wq = scale * ln_q_w[:, None] * f("wq")
    bq = scale * (ln_q_b @ f("wq") + f("bq"))
    wk = ln_kv_w[:, None] * f("wk"); bk = ln_kv_b @ f("wk") + f("bk")
    wv = ln_kv_w[:, None] * f("wv"); bv = ln_kv_b @ f("wv") + f("bv")
    w1 = ln_f_w[:, None] * f("w1"); b1 = ln_f_b @ f("w1") + f("b1")
    wo = f("wo"); bo = f("bo"); w2 = f("w2"); b2 = f("b2")

    # host-side: exp of the gated bias sum, transposed to [S, K] bf16
    et_host = []
    for b in range(B):
        cb = (c1 * ab[b] + c2 * ob[b]) * gate[b][:, None]   # [K, S]
        et_host.append(np.ascontiguousarray(
            np.exp(cb.T).astype(ml_dtypes.bfloat16)))        # [S, K]

    cont = np.ascontiguousarray
    bf = lambda a: np.ascontiguousarray(np.asarray(a, dtype=ml_dtypes.bfloat16))
    in_maps = []
    row_maps = []
    for c in range(N_CORES):
        b, r = divmod(c, 4)
        hs = slice(r * DH, (r + 1) * DH)
        rows = np.r_[r * P:(r + 1) * P, 512 + r * P:512 + (r + 1) * P]
        row_maps.append((b, rows))
        in_maps.append({
            "q": cont(q[b]), "kv": cont(kv[b]),
            "et": et_host[b],
            "gate": cont(gate[b][:, None]),
            "growb": bf(gate[b][None, :]),
            "q_res": cont(q[b][rows] + bo[None, :]),
            "wq": bf(wq[:, hs]), "wk": bf(wk[:, hs]), "wv": bf(wv[:, hs]),
            "bq": bf(bq[None, hs]), "bk": bf(bk[None, hs]),
            "bv": bf(bv[None, hs]),
            "wo": bf(wo[hs, :]), "w1": bf(w1), "b1": bf(b1[None, :]),
            "w2": bf(w2), "b2": bf(b2[None, :]),
        })

    global _last_in_maps
    _last_in_maps = in_maps
    nc = _get_nc()
    res = bass_utils.run_bass_kernel_spmd(
        nc, in_maps, core_ids=list(range(N_CORES)))
    out = np.empty((B, K, D), np.float32)
    for c in range(N_CORES):
        b, rows = row_maps[c]
        out[b][rows] = res.results[c]["xq"]
    return out


# revision 9
# speedup vs baseline: 1.7573x; 1.0661x over previous
"""CrossAttnBlock on 8 trn2 NeuronCores.

Sharding: core c -> batch b=c//4, rank r=c%4 within the batch group.
Attention is Megatron-sliced over heads (4 of 16 per core); the
out-projection partial sums are combined with TWO chunked bf16
ReduceScatters (one per K-half), each launched as soon as its half of
the attention context is ready, so the first fully overlaps the second
half of attention and the FFN on the first half overlaps the second RS.
After the RS, core r owns k-rows {r*128..+128} of each half; the host
maps them back.

Key optimizations vs the single-RS baseline:
- The combined attention bias term exp(gate*(c1*ab + c2*ob)) is computed
  on the host and streamed in as a [S, K] bf16 tensor (8.4 MB vs 33.6 MB
  f32 + on-device exp + transpose), block-streamed inside the attention
  loop, so the 320 us serial bias phase is gone entirely.
- QK^T matmuls for the two heads of a pair run concurrently in the PE
  array via tile_position row-packing (contraction is only 64).
- Softmax exp runs at FD=1024 over two PSUM banks per ACTIVATE.
- DMA queues are split (gpsimd: bulk loads, sync: transposes + stores,
  scalar: bias stream + tail weights) to avoid head-of-line blocking.
"""
import sys
import numpy as np

sys.path.insert(0, "/opt/trn_rl_repo")

import ml_dtypes  # noqa: E402
import concourse.bass as bass  # noqa: E402
import concourse.mybir as mybir  # noqa: E402
import concourse.tile as tile  # noqa: E402
from concourse import bacc  # noqa: E402
from concourse import bass_utils  # noqa: E402

F32 = mybir.dt.float32
BF16 = mybir.dt.bfloat16
AF = mybir.ActivationFunctionType
OP = mybir.AluOpType

D = 1024
H = 16
HD = 64
B = 2
K = 1024
S = 4096
EPS = 1e-5
N_CORES = 8
KQ = K // 4          # rows per core after the two ReduceScatters
HC = 4               # heads per core
DH = HC * HD         # ctx dims per core (256)
P = 128
DC = D // P          # 8 D-chunks
D2 = 2 * D

_CACHE = {}


def _ln_tile(nc, pool, xt, n_free, gate_col=None):
    """LN over free dim of xt [128, n_free] (f32 or bf16) -> bf16 tile.
    inv-std = exp(-0.5*log(var+eps)) runs on the (idle) scalar engine;
    gate_col: optional [128,1] f32 to fold into the scale."""
    n_sub = (n_free + 511) // 512
    st = pool.tile([P, n_sub, 6], F32, tag="ln_st")
    xs = xt.rearrange("p (s f) -> p s f", s=n_sub)
    for i in range(n_sub):
        nc.vector.bn_stats(out=st[:, i, :], in_=xs[:, i, :])
    mv = pool.tile([P, 2], F32, tag="ln_mv")
    nc.vector.bn_aggr(out=mv, in_=st[:, :, :])
    lv = pool.tile([P, 1], F32, tag="ln_lv")
    nc.scalar.activation(lv, mv[:, 1:2], AF.Ln, bias=nc._eps_t[:, :], scale=1.0)
    rs = pool.tile([P, 1], F32, tag="ln_rs")
    nc.scalar.activation(rs, lv, AF.Exp, scale=-0.5)
    if gate_col is not None:
        nc.vector.tensor_tensor(out=rs, in0=rs, in1=gate_col, op=OP.mult)
    xn = pool.tile([P, n_free], BF16, tag="ln_out")
    nc.vector.tensor_scalar(out=xn, in0=xt, scalar1=mv[:, 0:1], scalar2=rs,
                            op0=OP.subtract, op1=OP.mult)
    return xn


def _build_nc():
    nc = bacc.Bacc("TRN2", target_bir_lowering=False, debug=False,
                   num_devices=N_CORES)

    def din(name, shape, dt=F32):
        return nc.dram_tensor(name, shape, dt, kind="ExternalInput")

    q_d = din("q", [K, D])
    kv_d = din("kv", [S, D])
    et_d = din("et", [S, K], BF16)
    gate_d = din("gate", [K, 1])
    qres_d = din("q_res", [KQ, D])
    wq_d = din("wq", [D, DH], BF16)
    wk_d = din("wk", [D, DH], BF16)
    wv_d = din("wv", [D, DH], BF16)
    bq_d = din("bq", [1, DH], BF16)
    bk_d = din("bk", [1, DH], BF16)
    bv_d = din("bv", [1, DH], BF16)
    grow_d = din("growb", [1, K], BF16)
    wo_d = din("wo", [DH, D], BF16)
    w1_d = din("w1", [D, D2], BF16)
    b1_d = din("b1", [1, D2], BF16)
    w2_d = din("w2", [D2, D], BF16)
    b2_d = din("b2", [1, D], BF16)
    out_d = nc.dram_tensor("xq", [KQ, D], F32, kind="ExternalOutput")

    rs_out = [nc.dram_tensor(f"rs_out{i}", [P, D], BF16) for i in range(2)]
    groups = [[0, 1, 2, 3], [4, 5, 6, 7]]

    with tile.TileContext(nc) as tc:
        with (
            tc.tile_pool(name="const", bufs=1) as cpool,
            tc.tile_pool(name="persist", bufs=1) as pp,
            tc.tile_pool(name="wt", bufs=1) as wt,
            tc.tile_pool(name="dram", bufs=1, space="DRAM") as dpool,
        ):
            # ---- constants ----
            eps_t = cpool.tile([P, 1], F32)
            nc.vector.memset(eps_t, EPS)
            nc._eps_t = eps_t
            ones_row = cpool.tile([1, 512], BF16)
            nc.vector.memset(ones_row, 1.0)
            ones64 = cpool.tile([P, 64], BF16)
            nc.vector.memset(ones64, 1.0)
            dall = cpool.tile([97, 512], BF16)
            nc.vector.memset(dall, 1.0)
            gsb = cpool.tile([P, DC], F32)
            nc.sync.dma_start(out=gsb, in_=gate_d.ap().rearrange(
                "(t p) o -> p (t o)", p=P))
            grow_bf = cpool.tile([1, K], BF16)
            nc.sync.dma_start(out=grow_bf, in_=grow_d[:, :])
            bq_bf = cpool.tile([1, DH], BF16)
            nc.sync.dma_start(out=bq_bf, in_=bq_d[:, :])
            bk_bf = cpool.tile([1, DH], BF16)
            nc.sync.dma_start(out=bk_bf, in_=bk_d[:, :])
            bv_bf = cpool.tile([1, DH], BF16)
            nc.sync.dma_start(out=bv_bf, in_=bv_d[:, :])
            b1_bf = cpool.tile([1, D2], BF16)
            nc.sync.dma_start(out=b1_bf, in_=b1_d[:, :])
            b2_bf = cpool.tile([1, D], BF16)
            nc.sync.dma_start(out=b2_bf, in_=b2_d[:, :])

            # ---- persistent activation tensors ----
            qpT = pp.tile([P, 2, K], BF16)       # [2 heads x 64, hp, k]
            kpT = pp.tile([P, 2, S], BF16)
            vp = pp.tile([P, 32, HC * 65], BF16)  # [s%128, s//128, h*65+(hd|one)]
            ctxT = pp.tile([P, 2, K], BF16)

            # ============ q: LN+gate -> transpose -> projection ==========
            with (
                tc.tile_pool(name="projw", bufs=1) as wpool,
                tc.tile_pool(name="psA", bufs=4, space="PSUM") as psA,
            ):
                wq_bf = wpool.tile([P, DC, DH], BF16)
                nc.sync.dma_start(out=wq_bf, in_=wq_d.ap().rearrange(
                    "(c p) n -> p c n", p=P))
                wk_bf = wpool.tile([P, DC, DH], BF16)
                nc.sync.dma_start(out=wk_bf, in_=wk_d.ap().rearrange(
                    "(c p) n -> p c n", p=P))
                wv_bf = wpool.tile([P, DC, DH], BF16)
                nc.sync.dma_start(out=wv_bf, in_=wv_d.ap().rearrange(
                    "(c p) n -> p c n", p=P))

                with (
                    tc.tile_pool(name="lnq_big", bufs=1) as qbig,
                    tc.tile_pool(name="lnq", bufs=3) as lpool,
                ):
                    qT = qbig.tile([P, DC, K], BF16, tag="qT")
                    for t in range(K // P):
                        qt = lpool.tile([P, D], BF16, tag="ln_in")
                        nc.gpsimd.dma_start(out=qt,
                                            in_=q_d[t * P:(t + 1) * P, :])
                        qn = _ln_tile(nc, lpool, qt, D, gate_col=gsb[:, t:t + 1])
                        nc.sync.dma_start_transpose(
                            qT[:, :, t * P:(t + 1) * P], qn[:, :])
                    # q projection: psum[2hd, 512 tok]
                    for hp in range(2):
                        for tb in range(K // 512):
                            ps = psA.tile([P, 512], F32, tag="mm")
                            for dc in range(DC):
                                nc.tensor.matmul(
                                    ps[:, :],
                                    wq_bf[:, dc, hp * P:(hp + 1) * P],
                                    qT[:, dc, tb * 512:(tb + 1) * 512],
                                    start=(dc == 0), stop=False)
                            nc.tensor.matmul(
                                ps[:, :], bq_bf[0:1, hp * P:(hp + 1) * P],
                                grow_bf[0:1, tb * 512:(tb + 1) * 512],
                                start=False, stop=True)
                            nc.scalar.activation(
                                qpT[:, hp, tb * 512:(tb + 1) * 512], ps[:, :],
                                AF.Identity)

                # ====== kv: LN -> transpose -> k/v projections, by sg ======
                with (
                    tc.tile_pool(name="lnkv_big", bufs=2) as kbig,
                    tc.tile_pool(name="lnkv", bufs=3) as lpool,
                    tc.tile_pool(name="vps_p", bufs=2) as vpsp,
                ):
                    for h in range(HC):
                        nc.vector.memset(vp[:, :, h * 65 + 64:h * 65 + 65], 1.0)
                    for sg in range(4):
                        kvT = kbig.tile([P, DC, 1024], BF16, tag="kvT")
                        vpT = kbig.tile([P, 2, 1024], BF16, tag="vpT")
                        for t in range(8):
                            st_ = sg * 1024 + t * P
                            xt = lpool.tile([P, D], BF16, tag="ln_in")
                            nc.gpsimd.dma_start(out=xt, in_=kv_d[st_:st_ + P, :])
                            xn = _ln_tile(nc, lpool, xt, D)
                            nc.sync.dma_start_transpose(
                                kvT[:, :, t * P:(t + 1) * P], xn[:, :])
                        for hp in range(2):
                            for sb_ in range(2):
                                ssl = slice(sb_ * 512, (sb_ + 1) * 512)
                                for dst, w_bf, b_bf in ((kpT, wk_bf, bk_bf),
                                                        (vpT, wv_bf, bv_bf)):
                                    ps = psA.tile([P, 512], F32, tag="mm")
                                    for dc in range(DC):
                                        nc.tensor.matmul(
                                            ps[:, :],
                                            w_bf[:, dc, hp * P:(hp + 1) * P],
                                            kvT[:, dc, ssl],
                                            start=(dc == 0), stop=False)
                                    nc.tensor.matmul(
                                        ps[:, :],
                                        b_bf[0:1, hp * P:(hp + 1) * P],
                                        ones_row[0:1, :],
                                        start=False, stop=True)
                                    if dst is kpT:
                                        osl = slice(sg * 1024 + sb_ * 512,
                                                    sg * 1024 + (sb_ + 1) * 512)
                                        nc.scalar.activation(
                                            kpT[:, hp, osl], ps[:, :],
                                            AF.Identity)
                                    else:
                                        nc.scalar.activation(
                                            vpT[:, hp, ssl], ps[:, :],
                                            AF.Identity)
                        # vp natural layout [s, hd] per head (+ ones column)
                        for hp in range(2):
                            vps = vpsp.tile([P, 8, P], BF16, tag="vps")
                            nc.sync.dma_start_transpose(vps[:, :, :],
                                                        vpT[:, hp, :])
                            for half in range(2):
                                h = hp * 2 + half
                                nc.vector.tensor_copy(
                                    vp[:, sg * 8:(sg + 1) * 8,
                                       h * 65:h * 65 + 64],
                                    vps[:, :, half * 64:half * 64 + 64])

            # tail weights: gpsimd queue is idle once q/kv loads are done;
            # these loads overlap the start of attention.
            wo_bf = wt.tile([P, 2, D], BF16)
            nc.gpsimd.dma_start(out=wo_bf, in_=wo_d.ap().rearrange(
                "(c p) n -> p c n", p=P))
            w1_bf = wt.tile([P, DC, D2], BF16)
            nc.gpsimd.dma_start(out=w1_bf, in_=w1_d.ap().rearrange(
                "(c p) n -> p c n", p=P))
            w2_bf = wt.tile([P, D2 // P, D], BF16)
            nc.gpsimd.dma_start(out=w2_bf, in_=w2_d.ap().rearrange(
                "(c p) n -> p c n", p=P))
            qres_sb = wt.tile([P, 2, D], F32)
            nc.gpsimd.dma_start(out=qres_sb, in_=qres_d.ap().rearrange(
                "(t p) d -> p t d", p=P))

            # ======================= attention ==========================
            # loop kb (k halves) -> sc (s tiles) -> hp (head pairs);
            # the bias-exp block streams from HBM per (kb, sc).  After each
            # kb, the out-proj partial for that k-half is computed and its
            # ReduceScatter launched (overlapping the next kb / the FFN).
            with (
                tc.tile_pool(name="att", bufs=4) as apool,
                tc.tile_pool(name="ets", bufs=4) as espool,
                tc.tile_pool(name="attr", bufs=2) as rpool,
                tc.tile_pool(name="ysb", bufs=1) as ypool,
                tc.tile_pool(name="psS", bufs=2, space="PSUM") as psS,
                tc.tile_pool(name="psPV", bufs=1, space="PSUM") as psPV,
            ):
                for kb in range(K // 512):
                    ksl = slice(kb * 512, (kb + 1) * 512)
                    pvs = [psPV.tile([65, 512], F32, tag=f"pv{h}",
                                     name=f"pv_{kb}_{h}")
                           for h in range(HC)]
                    for sc in range(S // P):
                        et_blk = espool.tile([P, 512], BF16, tag="et")
                        nc.sync.dma_start(
                            out=et_blk,
                            in_=et_d.ap()[sc * P:(sc + 1) * P, ksl])
                        for hp in range(2):
                            sps = psS.tile([P, 1024], F32, tag="sc")
                            nc.tensor.matmul(
                                sps[:, 0:512],
                                kpT[0:64, hp, sc * P:(sc + 1) * P],
                                qpT[0:64, hp, ksl],
                                start=True, stop=True, tile_position=(0, 0))
                            nc.tensor.matmul(
                                sps[:, 512:1024],
                                kpT[64:128, hp, sc * P:(sc + 1) * P],
                                qpT[64:128, hp, ksl],
                                start=True, stop=True, tile_position=(64, 0))
                            eq = apool.tile([P, 1024], BF16, tag="eq")
                            nc.scalar.activation(eq, sps[:, :], AF.Exp)
                            at = apool.tile([P, 1024], BF16, tag="at")
                            et_v = et_blk[:, :].rearrange(
                                "p (o f) -> p o f", o=1).broadcast_to(
                                [P, 2, 512])
                            nc.vector.tensor_tensor(
                                out=at[:, :].rearrange("p (o f) -> p o f", o=2),
                                in0=eq[:, :].rearrange("p (o f) -> p o f", o=2),
                                in1=et_v, op=OP.mult)
                            he = hp * 2
                            ho = hp * 2 + 1
                            nc.tensor.matmul(
                                pvs[he][:, :],
                                vp[:, sc, he * 65:(he + 1) * 65],
                                at[:, 0:512],
                                start=(sc == 0), stop=(sc == S // P - 1))
                            nc.tensor.matmul(
                                pvs[ho][:, :],
                                vp[:, sc, ho * 65:(ho + 1) * 65],
                                at[:, 512:1024],
                                start=(sc == 0), stop=(sc == S // P - 1))
                    # denominators: spread the 4 heads onto partitions
                    # {0,32,64,96} so one reciprocal covers all of them.
                    for h in range(HC):
                        nc.vector.tensor_copy(dall[32 * h:32 * h + 1, :],
                                              pvs[h][64:65, :])
                    rden = rpool.tile([97, 512], BF16, tag="rden")
                    with nc.allow_low_precision(
                            reason="softmax denom reciprocal in bf16; "
                                   "ctx is bf16 anyway"):
                        nc.vector.reciprocal(rden, dall[:, :])
                    for h in range(HC):
                        pv = pvs[h]
                        pb = (h % 2) * 64
                        hp = h // 2
                        rps = psS.tile([P, 1024], F32, tag="sc")
                        nc.tensor.matmul(rps[0:64, 0:512],
                                         ones64[32 * h:32 * h + 1, :],
                                         rden[32 * h:32 * h + 1, :],
                                         start=True, stop=True,
                                         tile_position=(32 * h, 0))
                        rrs = rpool.tile([64, 512], BF16, tag="rrs")
                        nc.scalar.activation(rrs, rps[0:64, 0:512], AF.Identity)
                        nc.vector.tensor_tensor(
                            out=ctxT[pb:pb + 64, hp, ksl],
                            in0=pv[0:64, :], in1=rrs, op=OP.mult)
                    # ---- out-proj partial for this k-half + ReduceScatter
                    y_sb = ypool.tile([P, 4, D], BF16, tag="y")
                    for tb in range(4):
                        tsl = slice(kb * 512 + tb * P, kb * 512 + (tb + 1) * P)
                        for db in range(D // 512):
                            dsl = slice(db * 512, (db + 1) * 512)
                            ps = psS.tile([P, 1024], F32, tag="sc")
                            for cc in range(2):
                                nc.tensor.matmul(
                                    ps[:, 0:512],
                                    ctxT[:, cc, tsl],
                                    wo_bf[:, cc, dsl],
                                    start=(cc == 0), stop=(cc == 1))
                            nc.vector.tensor_copy(y_sb[:, tb, dsl],
                                                  ps[:, 0:512])
                    rs_in = dpool.tile([512, D], BF16, tag=f"rsin{kb}")
                    nc.sync.dma_start(
                        out=rs_in.rearrange("(t p) d -> p t d", p=P),
                        in_=y_sb[:, :, :])
                    nc.gpsimd.collective_compute(
                        "ReduceScatter", OP.add, replica_groups=groups,
                        ins=[rs_in.opt()], outs=[rs_out[kb].ap().opt()])

            # ====== residual + LN_f + FFN per k-half (kt0 under RS1) ====
            with (
                tc.tile_pool(name="ffn", bufs=1) as fp,
                tc.tile_pool(name="fstream", bufs=3) as fs,
                tc.tile_pool(name="psF", bufs=3, space="PSUM") as psF,
                tc.tile_pool(name="psH", bufs=1, space="PSUM") as psH,
            ):
                x_sb = fp.tile([P, 2, D], F32)
                xfT = fp.tile([P, DC, KQ], BF16)
                h1T = fp.tile([P, D2 // P, KQ], BF16)
                o_sb = fp.tile([P, 2, D], F32)
                for kt in range(2):
                    rs_sb = fs.tile([P, D], BF16, tag="rs")
                    nc.scalar.dma_start(out=rs_sb, in_=rs_out[kt].ap())
                    nc.vector.tensor_tensor(out=x_sb[:, kt, :], in0=rs_sb,
                                            in1=qres_sb[:, kt, :], op=OP.add)
                    xn = _ln_tile(nc, fs, x_sb[:, kt, :], D)
                    nc.sync.dma_start_transpose(
                        xfT[:, :, kt * P:(kt + 1) * P], xn[:, :])
                    # FFN1 flipped: xfT chunks stationary, w1 streams
                    h1 = fs.tile([P, 4, 512], BF16, tag="h1")
                    for hb in range(4):
                        ps = psF.tile([P, 512], F32, tag="f")
                        hsl = slice(hb * 512, (hb + 1) * 512)
                        for dc in range(DC):
                            nc.tensor.matmul(
                                ps[:, :], xfT[:, dc, kt * P:(kt + 1) * P],
                                w1_bf[:, dc, hsl],
                                start=(dc == 0), stop=False)
                        nc.tensor.matmul(
                            ps[:, :], ones_row[0:1, 0:P],
                            b1_bf[0:1, hsl], start=False, stop=True)
                        nc.scalar.activation(h1[:, hb, :], ps[:, :], AF.Gelu)
                    nc.sync.dma_start_transpose(
                        h1T[:, :, kt * P:(kt + 1) * P], h1[:, :, :])
                    # FFN2: accumulate over hc chunks, both D halves live
                    ps0 = psH.tile([P, 512], F32, tag="o0")
                    ps1 = psH.tile([P, 512], F32, tag="o1")
                    for hc in range(D2 // P):
                        for db, ps in ((0, ps0), (1, ps1)):
                            nc.tensor.matmul(
                                ps[:, :], h1T[:, hc, kt * P:(kt + 1) * P],
                                w2_bf[:, hc, db * 512:(db + 1) * 512],
                                start=(hc == 0), stop=False)
                    for db, ps in ((0, ps0), (1, ps1)):
                        dsl = slice(db * 512, (db + 1) * 512)
                        nc.tensor.matmul(
                            ps[:, :], ones_row[0:1, 0:P],
                            b2_bf[0:1, dsl], start=False, stop=True)
                        nc.vector.tensor_tensor(out=o_sb[:, kt, dsl],
                                                in0=ps[:, :],
                                                in1=x_sb[:, kt, dsl],
                                                op=OP.add)
                    nc.sync.dma_start(
                        out=out_d.ap()[kt * P:(kt + 1) * P, :],
                        in_=o_sb[:, kt, :])

    nc.compile()
    return nc


def _get_nc():
    if "nc" not in _CACHE:
        _CACHE["nc"] = _build_nc()
    return _CACHE["nc"]


def _softplus(x):
    return float(np.log1p(np.exp(np.float64(x))))


def kernel(**inputs):
    f = lambda name: np.ascontiguousarray(np.asarray(inputs[name], np.float32))
    q = f("q"); kv = f("kv"); ab = f("attn_bias"); ob = f("obs_bias")
    density = f("density")
    c1 = _softplus(inputs["dist_raw"])
    c2 = _softplus(inputs["obs_raw"])
    tg = float(np.tanh(np.float64(np.asarray(inputs["dens_raw"], np.float64))))
    gate = (1.0 + tg * density).astype(np.float32)       # [B, K]

    ln_q_w = f("ln_q_w"); ln_q_b = f("ln_q_b")
    ln_kv_w = f("ln_kv_w"); ln_kv_b = f("ln_kv_b")
    ln_f_w = f("ln_f_w"); ln_f_b = f("ln_f_b")
    scale = np.float32(HD ** -0.5)
    wq = scale * ln_q_w[:, None] * f("wq")
    bq = scale * (ln_q_b @ f("wq") + f("bq"))
    wk = ln_kv_w[:, None] * f("wk"); bk = ln_kv_b @ f("wk") + f("bk")
    wv = ln_kv_w[:, None] * f("wv"); bv = ln_kv_b @ f("wv") + f("bv")
    w1 = ln_f_w[:, None] * f("w1"); b1 = ln_f_b @ f("w1") + f("b1")
    wo = f("wo"); bo = f("bo"); w2 = f("w2"); b2 = f("b2")

    # host-side: exp of the gated bias sum, transposed to [S, K] bf16
    et_host = []
    for b in range(B):
        cb = (c1 * ab[b] + c2 * ob[b]) * gate[b][:, None]   # [K, S]
        et_host.append(np.ascontiguousarray(
            np.exp(cb.T).astype(ml_dtypes.bfloat16)))        # [S, K]

    cont = np.ascontiguousarray
    bf = lambda a: np.ascontiguousarray(np.asarray(a, dtype=ml_dtypes.bfloat16))
    in_maps = []
    row_maps = []
    for c in range(N_CORES):
        b, r = divmod(c, 4)
        hs = slice(r * DH, (r + 1) * DH)
        rows = np.r_[r * P:(r + 1) * P, 512 + r * P:512 + (r + 1) * P]
        row_maps.append((b, rows))
        in_maps.append({
            "q": cont(q[b]), "kv": cont(kv[b]),
            "et": et_host[b],
            "gate": cont(gate[b][:, None]),
            "growb": bf(gate[b][None, :]),
            "q_res": cont(q[b][rows] + bo[None, :]),
            "wq": bf(wq[:, hs]), "wk": bf(wk[:, hs]), "wv": bf(wv[:, hs]),
            "bq": bf(bq[None, hs]), "bk": bf(bk[None, hs]),
            "bv": bf(bv[None, hs]),
            "wo": bf(wo[hs, :]), "w1": bf(w1), "b1": bf(b1[None, :]),
            "w2": bf(w2), "b2": bf(b2[None, :]),
        })

    global _last_in_maps
    _last_in_maps = in_maps
    nc = _get_nc()
    res = bass_utils.run_bass_kernel_spmd(
        nc, in_maps, core_ids=list(range(N_CORES)))
    out = np.empty((B, K, D), np.float32)
    for c in range(N_CORES):
        b, rows = row_maps[c]
        out[b][rows] = res.results[c]["xq"]
    return out
